# revision 13
# baseline (speedup 1.0000x reference)
"""DKFN (graph-conv LSTM cell) Trainium2 kernel, data-parallel over batch on 8 cores.

Contract: kernel(**inputs) takes the full unsharded inputs (numpy, fp32) and
returns the full outputs (tuple matching the reference). All compute runs on
8 NeuronCores via bass/Tile; host only shards/transposes/gathers.

Layout strategy: everything on device lives "transposed" ([features, batch])
so both matmul operands always have the contraction axis on partitions and no
on-device transposes are needed. Host pre-transposes inputs and weights and
post-transposes outputs.
"""

import sys

sys.path.insert(0, "/opt/trn_rl_repo")

import numpy as np

import concourse.bass as bass
import concourse.bacc as bacc
import concourse.tile as tile
import concourse.mybir as mybir
from concourse import bass_utils
from concourse.bass import ts

NCORES = 8
B, N, K = 8192, 1024, 3
Bs = B // NCORES          # 1024 batch rows per core
P = 128
NT = N // P               # 8 feature tiles of 128
GJT = (K + 1) * N // P    # 32 contraction tiles for main gates
RJT = 2 * N // P          # 16 contraction tiles for r-gates
M1_CNT = B * N            # var1 element count
M2_CNT = B * K * N        # var2 element count

F32 = mybir.dt.float32

# Matmul dtype config: "f32", "f32r" (bitcast to float32r at matmul only),
# or "bf16" (activations+weights cast to bf16; psum stays fp32).
MM_MODE = "f32"


def _mm_dt():
    return mybir.dt.bfloat16 if MM_MODE == "bf16" else F32


def _mm_cast(ap):
    """View an fp32 SBUF AP as float32r for the PE when in f32r mode."""
    if MM_MODE == "f32r":
        return ap.bitcast(mybir.dt.float32r)
    return ap


FREE = 512 if MM_MODE != "bf16" else 1024   # moving-operand free dim
BH = Bs // FREE                             # batch halves per 1024


def _build():
    mm_dt = _mm_dt()
    nc = bacc.Bacc("TRN2", target_bir_lowering=False, debug=False,
                   num_devices=NCORES)

    # ---- DRAM I/O ----
    d_xT = nc.dram_tensor("xT", [N, Bs], mm_dt, kind="ExternalInput").ap()
    d_hT = nc.dram_tensor("hT", [N, Bs], mm_dt, kind="ExternalInput").ap()
    d_rhT = nc.dram_tensor("rhT", [N, Bs], mm_dt, kind="ExternalInput").ap()
    d_cT = nc.dram_tensor("cT", [N, Bs], F32, kind="ExternalInput").ap()
    d_rcT = nc.dram_tensor("rcT", [N, Bs], F32, kind="ExternalInput").ap()
    d_A = nc.dram_tensor("A", [N, N], F32, kind="ExternalInput").ap()
    d_AT = nc.dram_tensor("AT", [N, N], F32, kind="ExternalInput").ap()
    d_wgcT = nc.dram_tensor("wgcT", [K, N, N], F32, kind="ExternalInput").ap()
    # gate weights, host-tiled: [mb, jt, gate, 128, 128] (lhsT blocks)
    d_w4 = nc.dram_tensor("w4", [NT, GJT, 4, P, P], mm_dt,
                          kind="ExternalInput").ap()
    d_rw4 = nc.dram_tensor("rw4", [NT, RJT, 4, P, P], mm_dt,
                           kind="ExternalInput").ap()
    d_bias = nc.dram_tensor("bias", [8, NT, P], F32, kind="ExternalInput").ap()
    d_nw = nc.dram_tensor("nw", [N, 1], F32, kind="ExternalInput").ap()
    d_c = nc.dram_tensor("c", [1, 1], F32, kind="ExternalInput").ap()

    d_HT = nc.dram_tensor("HT", [N, Bs], F32, kind="ExternalOutput").ap()
    d_CT = nc.dram_tensor("CT", [N, Bs], F32, kind="ExternalOutput").ap()
    d_gcT = nc.dram_tensor("gcT", [K * N, Bs], F32, kind="ExternalOutput").ap()
    d_rHT = nc.dram_tensor("rHT", [N, Bs], F32, kind="ExternalOutput").ap()
    d_rCT = nc.dram_tensor("rCT", [N, Bs], F32, kind="ExternalOutput").ap()
    d_predT = nc.dram_tensor("predT", [N, Bs], F32, kind="ExternalOutput").ap()

    AF = mybir.ActivationFunctionType
    ALU = mybir.AluOpType
    AX = mybir.AxisListType

    with tile.TileContext(nc) as tc:
        with (
            tc.tile_pool(name="persist", bufs=1) as pp,
            tc.tile_pool(name="dram", bufs=1, space="DRAM") as dram,
        ):
            # persistent small tiles
            ncv = pp.tile([P, NT], F32, name="ncv")           # A3 @ nw, col=nb
            ssum = pp.tile([P, NT + K * NT], F32, name="ssum")  # per-tile sums
            ssq = pp.tile([P, NT + K * NT], F32, name="ssq")    # per-tile sumsqs
            bias_sb = pp.tile([P, 8 * NT], F32, name="bias_sb")  # [g*8+mb]
            for g in range(8):
                for t in range(NT):
                    nc.sync.dma_start(bias_sb[:, g * NT + t:g * NT + t + 1],
                                      d_bias[g, t][:, None])
            # DRAM scratch: filters (transposed), mm dtype
            filt_d = dram.tile([K, NT, P, N], mm_dt, name="filt_d")
            # collective bounce buffers
            cin = dram.tile([1, 4], F32, name="cin")
            cout = dram.tile([1, 4], F32, name="cout", addr_space="Shared")

            # =========== Stage A: A-matrix prep ===========
            with (
                tc.tile_pool(name="abig", bufs=1) as ab,
                tc.tile_pool(name="asmall", bufs=1) as asm,
                tc.tile_pool(name="astream", bufs=3) as ast,
                tc.tile_pool(name="apsum", bufs=4, space="PSUM") as aps,
            ):
                # three big [P, NT, N] slots, sequenced by tag reuse:
                #   mat0: at_sb -> m2;  mat1: m1 -> m3;  mat2: a_sb
                a_sb = ab.tile([P, NT, N], F32, name="a_sb", tag="mat2")
                at_sb = ab.tile([P, NT, N], F32, name="at_sb", tag="mat0")
                nc.sync.dma_start(a_sb[:], d_A.rearrange("(t p) j -> p t j", p=P))
                nc.sync.dma_start(at_sb[:], d_AT.rearrange("(t p) j -> p t j", p=P))

                # col-sums of A as columns (= row-sums of A^T)
                cs = asm.tile([P, NT], F32, name="cs")
                for t in range(NT):
                    nc.vector.tensor_reduce(cs[:, t:t + 1], at_sb[:, t, :],
                                            axis=AX.X, op=ALU.add)
                rc_col = asm.tile([P, NT], F32, name="rc_col")
                nc.vector.reciprocal(rc_col[:], cs[:])

                # col-sums of A as a row (ones^T @ A), then reciprocal+bcast
                ones = asm.tile([P, 1], F32, name="ones")
                nc.gpsimd.memset(ones[:], 1.0)
                ps_cs = aps.tile([1, N], F32, name="ps_cs", bufs=1)
                for h in range(N // 512):
                    for t in range(NT):
                        nc.tensor.matmul(ps_cs[:, ts(h, 512)], ones[:],
                                         a_sb[:, t, ts(h, 512)],
                                         start=(t == 0), stop=(t == NT - 1))
                rcr = asm.tile([1, N], F32, name="rcr")
                nc.vector.reciprocal(rcr[:], ps_cs[:])
                rc_bc = asm.tile([P, N], F32, name="rc_bc")
                nc.gpsimd.partition_broadcast(rc_bc[:], rcr[:])

                # M1 = clamp(A^T D)
                m1 = ab.tile([P, NT, N], F32, name="m1", tag="mat1")
                for t in range(NT):
                    nc.vector.tensor_mul(m1[:, t, :], at_sb[:, t, :], rc_bc[:])
                    nc.vector.tensor_scalar_min(m1[:, t, :], m1[:, t, :], 1.0)

                def hop(m_prev, m_next):
                    # m_next = clamp(A^T @ (D m_prev)), staging D@m_prev by
                    # batch-half to save SBUF
                    for h in range(N // 512):
                        dmh = ab.tile([P, NT, 512], F32, name="dmh", tag="dmh",
                                      bufs=2)
                        for t in range(NT):
                            nc.vector.tensor_scalar_mul(
                                dmh[:, t, :], m_prev[:, t, ts(h, 512)],
                                rc_col[:, t:t + 1])
                        for jb in range(NT):
                            ps = aps.tile([P, 512], F32, name="ps_hop",
                                          tag="ps_hop")
                            for lt in range(NT):
                                nc.tensor.matmul(ps[:], a_sb[:, lt, ts(jb, P)],
                                                 dmh[:, lt, :],
                                                 start=(lt == 0),
                                                 stop=(lt == NT - 1))
                            nc.vector.tensor_scalar_min(
                                m_next[:, jb, ts(h, 512)], ps[:], 1.0)

                def spill_filt(k, m_k):
                    for t in range(NT):
                        wg = ast.tile([P, N], F32, name="wg", tag="wg")
                        nc.sync.dma_start(wg[:], d_wgcT[k, ts(t, P), :])
                        f32t = ast.tile([P, N], F32, name="f32t", tag="f32t")
                        nc.vector.tensor_mul(f32t[:], m_k[:, t, :], wg[:])
                        if mm_dt != F32:
                            fmm = ast.tile([P, N], mm_dt, name="fmm", tag="fmm")
                            nc.vector.tensor_copy(fmm[:], f32t[:])
                            nc.sync.dma_start(filt_d[k, t], fmm[:])
                        else:
                            nc.sync.dma_start(filt_d[k, t], f32t[:])

                m2 = ab.tile([P, NT, N], F32, name="m2", tag="mat0")
                hop(m1, m2)
                spill_filt(0, m1)
                m3 = ab.tile([P, NT, N], F32, name="m3", tag="mat1")
                hop(m2, m3)
                spill_filt(1, m2)
                spill_filt(2, m3)

                # NC_vec = A3 @ nw  (per-feature column scalars)
                nw_sb = asm.tile([P, NT], F32, name="nw_sb")
                nc.sync.dma_start(nw_sb[:],
                                  d_nw.rearrange("(t p) o -> p (t o)", p=P))
                ps_ncv = aps.tile([P, NT], F32, name="ps_ncv", bufs=1)
                for nb in range(NT):
                    for lt in range(NT):
                        nc.tensor.matmul(ps_ncv[:, nb:nb + 1],
                                         m3[:, lt, ts(nb, P)],
                                         nw_sb[:, lt:lt + 1],
                                         start=(lt == 0), stop=(lt == NT - 1))
                nc.vector.tensor_copy(ncv[:], ps_ncv[:])

            # =========== Stage B: r-gates (rf, ri, ro, rC) ===========
            # rcombined^T = [xT; rhT], contraction over 2N
            xh_pool = tc.tile_pool(name="xh", bufs=1, side="right")
            xh = xh_pool.__enter__()
            x_sb = xh.tile([P, NT, Bs], mm_dt, name="x_sb")
            nc.sync.dma_start(x_sb[:], d_xT.rearrange("(t p) b -> p t b", p=P))
            # input stats (var1) off the mm-dtype copy
            with tc.tile_pool(name="xstat", bufs=2) as xst:
                for t in range(NT):
                    nc.vector.tensor_reduce(ssum[:, t:t + 1], x_sb[:, t, :],
                                            axis=AX.X, op=ALU.add)
                    sq = xst.tile([P, Bs], F32, name="sq", tag="sq")
                    nc.scalar.activation(sq[:], x_sb[:, t, :], AF.Square,
                                         accum_out=ssq[:, t:t + 1])

            with (
                tc.tile_pool(name="rh", bufs=1) as rhp,
                tc.tile_pool(name="rwst", bufs=4) as rwst,
                tc.tile_pool(name="rgo", bufs=2) as rgo,
                tc.tile_pool(name="rcst", bufs=2) as rcst,
                tc.tile_pool(name="rpsum", bufs=4 * BH, space="PSUM") as rps,
            ):
                rh_sb = rhp.tile([P, NT, Bs], mm_dt, name="rh_sb")
                nc.sync.dma_start(rh_sb[:],
                                  d_rhT.rearrange("(t p) b -> p t b", p=P))

                def rrhs(jt):
                    return (x_sb[:, jt, :] if jt < NT
                            else rh_sb[:, jt - NT, :])

                for mb in range(NT):
                    pss = [[rps.tile([P, FREE], F32, name="rps", tag="rps")
                            for _ in range(BH)] for _ in range(4)]
                    for jt in range(RJT):
                        w4t = rwst.tile([P, 4, P], mm_dt, name="w4t", tag="w4t")
                        nc.sync.dma_start(
                            w4t[:], d_rw4[mb, jt].rearrange("g p m -> p g m"))
                        for g in range(4):
                            for h in range(BH):
                                nc.tensor.matmul(
                                    pss[g][h][:], _mm_cast(w4t[:, g, :]),
                                    _mm_cast(rrhs(jt)[:, ts(h, FREE)]),
                                    start=(jt == 0), stop=(jt == RJT - 1))
                    # activations with bias: rf, ri, ro sigmoid; rC tanh
                    gt = [rgo.tile([P, Bs], F32, name=f"rg{g}", tag=f"rg{g}")
                          for g in range(4)]
                    for g in range(4):
                        fn = AF.Tanh if g == 3 else AF.Sigmoid
                        for h in range(BH):
                            nc.scalar.activation(
                                gt[g][:, ts(h, FREE)], pss[g][h][:], fn,
                                bias=bias_sb[:, (4 + g) * NT + mb:
                                             (4 + g) * NT + mb + 1])
                    # pointwise: rC_new = rf*rcT + ri*rCt ; rH = ro*tanh(rC_new)
                    rc_t = rcst.tile([P, Bs], F32, name="rc_t", tag="rc_t")
                    nc.sync.dma_start(rc_t[:], d_rcT[ts(mb, P), :])
                    t1 = rcst.tile([P, Bs], F32, name="t1", tag="t1")
                    nc.vector.tensor_mul(t1[:], gt[0][:], rc_t[:])
                    nc.vector.tensor_mul(rc_t[:], gt[1][:], gt[3][:])
                    nc.vector.tensor_add(rc_t[:], t1[:], rc_t[:])
                    nc.sync.dma_start(d_rCT[ts(mb, P), :], rc_t[:])
                    nc.scalar.activation(t1[:], rc_t[:], AF.Tanh)
                    nc.vector.tensor_mul(t1[:], gt[2][:], t1[:])
                    nc.sync.dma_start(d_rHT[ts(mb, P), :], t1[:])

            # =========== Stage C: gc = K-hop graph conv ===========
            gc_pool = tc.tile_pool(name="gc", bufs=1)
            gcp = gc_pool.__enter__()
            gc_sb = gcp.tile([P, K * NT, Bs], mm_dt, name="gc_sb")
            with (
                tc.tile_pool(name="fk", bufs=1) as fkp,
                tc.tile_pool(name="gdr", bufs=3) as gdr,
                tc.tile_pool(name="gpsum", bufs=3 * BH, space="PSUM") as gps,
            ):
                for k in range(K):
                    fk = fkp.tile([P, NT, N], mm_dt, name="fk", tag="fk")
                    nc.sync.dma_start(fk[:],
                                      filt_d[k].rearrange("t p m -> p t m"))
                    for mb in range(NT):
                        idx = k * NT + mb
                        pss = [gps.tile([P, FREE], F32, name="gps", tag="gps")
                               for _ in range(BH)]
                        for h in range(BH):
                            for lt in range(NT):
                                nc.tensor.matmul(
                                    pss[h][:], _mm_cast(fk[:, lt, ts(mb, P)]),
                                    _mm_cast(x_sb[:, lt, ts(h, FREE)]),
                                    start=(lt == 0), stop=(lt == NT - 1))
                        if mm_dt != F32:
                            drain = gdr.tile([P, Bs], F32, name="drain",
                                             tag="drain")
                        else:
                            drain = gc_sb[:, idx, :]
                        for h in range(BH):
                            nc.vector.tensor_copy(drain[:, ts(h, FREE)],
                                                  pss[h][:])
                        nc.sync.dma_start(d_gcT[ts(idx, P), :], drain[:])
                        # stats for var2
                        nc.vector.tensor_reduce(ssum[:, NT + idx:NT + idx + 1],
                                                drain[:], axis=AX.X, op=ALU.add)
                        sqg = gdr.tile([P, Bs], F32, name="sqg", tag="sqg")
                        nc.scalar.activation(sqg[:], drain[:], AF.Square,
                                             accum_out=ssq[:, NT + idx:
                                                           NT + idx + 1])
                        if mm_dt != F32:
                            nc.vector.tensor_copy(gc_sb[:, idx, :], drain[:])

            # kick off the variance all-reduce (overlaps with stage D)
            with tc.tile_pool(name="cstat", bufs=1) as cst:
                cin_sb = cst.tile([1, 4], F32, name="cin_sb")
                nc.gpsimd.tensor_reduce(cin_sb[:, 0:1], ssum[:, :NT],
                                        axis=AX.XYZWC, op=ALU.add)
                nc.gpsimd.tensor_reduce(cin_sb[:, 1:2], ssq[:, :NT],
                                        axis=AX.XYZWC, op=ALU.add)
                nc.gpsimd.tensor_reduce(cin_sb[:, 2:3], ssum[:, NT:],
                                        axis=AX.XYZWC, op=ALU.add)
                nc.gpsimd.tensor_reduce(cin_sb[:, 3:4], ssq[:, NT:],
                                        axis=AX.XYZWC, op=ALU.add)
                nc.sync.dma_start(cin[:], cin_sb[:])
                nc.gpsimd.collective_compute(
                    "AllReduce", mybir.AluOpType.add,
                    replica_groups=[list(range(NCORES))],
                    ins=[cin.opt()], outs=[cout.opt()])

            xh_pool.__exit__(None, None, None)  # release x_sb

            # =========== Stage D: main gates (f, i, o, C) ===========
            with (
                tc.tile_pool(name="hh", bufs=1) as hhp,
                tc.tile_pool(name="wst", bufs=4) as wst,
                tc.tile_pool(name="go", bufs=2) as gop,
                tc.tile_pool(name="cst2", bufs=2) as cst2,
                tc.tile_pool(name="dpsum", bufs=4 * BH, space="PSUM") as dps,
            ):
                h_sb = hhp.tile([P, NT, Bs], mm_dt, name="h_sb")
                nc.sync.dma_start(h_sb[:],
                                  d_hT.rearrange("(t p) b -> p t b", p=P))

                def grhs(jt):
                    return (gc_sb[:, jt, :] if jt < K * NT
                            else h_sb[:, jt - K * NT, :])

                for mb in range(NT):
                    pss = [[dps.tile([P, FREE], F32, name="dps", tag="dps")
                            for _ in range(BH)] for _ in range(4)]
                    for jt in range(GJT):
                        w4t = wst.tile([P, 4, P], mm_dt, name="gw4t", tag="gw4t")
                        nc.sync.dma_start(
                            w4t[:], d_w4[mb, jt].rearrange("g p m -> p g m"))
                        for g in range(4):
                            for h in range(BH):
                                nc.tensor.matmul(
                                    pss[g][h][:], _mm_cast(w4t[:, g, :]),
                                    _mm_cast(grhs(jt)[:, ts(h, FREE)]),
                                    start=(jt == 0), stop=(jt == GJT - 1))
                    gt = [gop.tile([P, Bs], F32, name=f"g{g}", tag=f"g{g}")
                          for g in range(4)]
                    for g in range(4):
                        fn = AF.Tanh if g == 3 else AF.Sigmoid
                        for h in range(BH):
                            nc.scalar.activation(
                                gt[g][:, ts(h, FREE)], pss[g][h][:], fn,
                                bias=bias_sb[:, g * NT + mb:g * NT + mb + 1])
                    # pointwise: NC = cT * ncv ; C_new = f*NC + i*Ct
                    c_t = cst2.tile([P, Bs], F32, name="c_t", tag="c_t")
                    nc.sync.dma_start(c_t[:], d_cT[ts(mb, P), :])
                    nc.vector.tensor_scalar_mul(c_t[:], c_t[:],
                                                ncv[:, mb:mb + 1])
                    nc.vector.tensor_mul(c_t[:], gt[0][:], c_t[:])
                    t2 = cst2.tile([P, Bs], F32, name="t2", tag="t2")
                    nc.vector.tensor_mul(t2[:], gt[1][:], gt[3][:])
                    nc.vector.tensor_add(c_t[:], c_t[:], t2[:])
                    nc.sync.dma_start(d_CT[ts(mb, P), :], c_t[:])
                    nc.scalar.activation(t2[:], c_t[:], AF.Tanh)
                    nc.vector.tensor_mul(t2[:], gt[2][:], t2[:])
                    nc.sync.dma_start(d_HT[ts(mb, P), :], t2[:])

            gc_pool.__exit__(None, None, None)

            # =========== Stage E: pred blend ===========
            with tc.tile_pool(name="ep", bufs=2) as ep:
                v = ep.tile([1, 4], F32, name="v", bufs=1)
                nc.sync.dma_start(v[:], cout[:])
                c_sc = ep.tile([1, 1], F32, name="c_sc", bufs=1)
                nc.sync.dma_start(c_sc[:], d_c[:])
                sc = ep.tile([1, 8], F32, name="sc", bufs=1)
                # var1 = (Q1 - S1^2/M1) / (M1-1); var2 likewise
                nc.vector.tensor_mul(sc[:, 0:1], v[:, 0:1], v[:, 0:1])
                nc.vector.tensor_scalar_mul(sc[:, 0:1], sc[:, 0:1],
                                            -1.0 / M1_CNT)
                nc.vector.tensor_add(sc[:, 0:1], sc[:, 0:1], v[:, 1:2])
                nc.vector.tensor_scalar_mul(sc[:, 0:1], sc[:, 0:1],
                                            1.0 / (M1_CNT - 1))
                nc.vector.tensor_mul(sc[:, 1:2], v[:, 2:3], v[:, 2:3])
                nc.vector.tensor_scalar_mul(sc[:, 1:2], sc[:, 1:2],
                                            -1.0 / M2_CNT)
                nc.vector.tensor_add(sc[:, 1:2], sc[:, 1:2], v[:, 3:4])
                nc.vector.tensor_scalar_mul(sc[:, 1:2], sc[:, 1:2],
                                            1.0 / (M2_CNT - 1))
                # alpha = var1*c/den, beta = var2/den, den = var1 + var2*c
                nc.vector.tensor_mul(sc[:, 2:3], sc[:, 0:1], c_sc[:])  # var1*c
                nc.vector.tensor_mul(sc[:, 3:4], sc[:, 1:2], c_sc[:])  # var2*c
                nc.vector.tensor_add(sc[:, 4:5], sc[:, 0:1], sc[:, 3:4])
                nc.vector.reciprocal(sc[:, 5:6], sc[:, 4:5])
                nc.vector.tensor_mul(sc[:, 6:7], sc[:, 2:3], sc[:, 5:6])  # a
                nc.vector.tensor_mul(sc[:, 7:8], sc[:, 1:2], sc[:, 5:6])  # b
                ab = ep.tile([P, 2], F32, name="ab", bufs=1)
                nc.gpsimd.partition_broadcast(ab[:], sc[:, 6:8])
                for t in range(NT):
                    h_t = ep.tile([P, Bs], F32, name="h_t", tag="h_t")
                    rh_t = ep.tile([P, Bs], F32, name="rh_t", tag="rh_t")
                    nc.sync.dma_start(h_t[:], d_HT[ts(t, P), :])
                    nc.sync.dma_start(rh_t[:], d_rHT[ts(t, P), :])
                    nc.vector.tensor_scalar_mul(rh_t[:], rh_t[:], ab[:, 1:2])
                    nc.vector.scalar_tensor_tensor(
                        h_t[:], h_t[:], ab[:, 0:1], rh_t[:],
                        op0=ALU.mult, op1=ALU.add)
                    nc.sync.dma_start(d_predT[ts(t, P), :], h_t[:])

    nc.compile()
    return nc


_NC_CACHE = {}


def _get_nc():
    if "nc" not in _NC_CACHE:
        _NC_CACHE["nc"] = _build()
    return _NC_CACHE["nc"]


def _np_mm(x):
    """Cast host array to the matmul wire dtype."""
    if MM_MODE == "bf16":
        import ml_dtypes
        return x.astype(ml_dtypes.bfloat16)
    return np.ascontiguousarray(x, dtype=np.float32)


def _tile_gate_weights(ws):
    """ws: list of 4 [N, J] arrays -> [NT, JT, 4, P, P] lhsT blocks.

    lhsT block (mb, jt, g) must be w_g^T[jt*P:(jt+1)*P, mb*P:(mb+1)*P],
    i.e. element [p, m] = w_g[mb*P+m, jt*P+p].
    """
    J = ws[0].shape[1]
    jt_n = J // P
    out = np.empty((NT, jt_n, 4, P, P), dtype=np.float32)
    for g, w in enumerate(ws):
        # w: [N, J] -> blocks [mb, jt, p, m] = w[mb*P+m, jt*P+p]
        blk = w.reshape(NT, P, jt_n, P).transpose(0, 2, 3, 1)
        out[:, :, g] = blk
    return out


def _prep_in_maps(inputs):
    input = inputs["input"]
    Hidden_State = inputs["Hidden_State"]
    Cell_State = inputs["Cell_State"]
    rHidden_State = inputs["rHidden_State"]
    rCell_State = inputs["rCell_State"]
    A = inputs["A"]
    W_gc = inputs["W_gc"]
    wf, wi, wo, wC = inputs["wf"], inputs["wi"], inputs["wo"], inputs["wC"]
    rwf, rwi, rwo, rwC = (inputs["rwf"], inputs["rwi"], inputs["rwo"],
                          inputs["rwC"])
    bf, bi, bo, bC = inputs["bf"], inputs["bi"], inputs["bo"], inputs["bC"]
    rbf, rbi, rbo, rbC = (inputs["rbf"], inputs["rbi"], inputs["rbo"],
                          inputs["rbC"])
    neighbor_weight, c = inputs["neighbor_weight"], inputs["c"]

    A32 = np.ascontiguousarray(A, dtype=np.float32)
    AT = np.ascontiguousarray(A32.T)
    wgcT = np.ascontiguousarray(np.transpose(W_gc, (0, 2, 1)), dtype=np.float32)
    w4 = _np_mm(_tile_gate_weights([wf, wi, wo, wC]))
    rw4 = _np_mm(_tile_gate_weights([rwf, rwi, rwo, rwC]))
    bias = np.stack([bf, bi, bo, bC, rbf, rbi, rbo, rbC]) \
        .astype(np.float32).reshape(8, NT, P)
    nw = np.ascontiguousarray(neighbor_weight, dtype=np.float32).reshape(N, 1)
    c_h = np.asarray(c, dtype=np.float32).reshape(1, 1)

    shared = {"A": A32, "AT": AT, "wgcT": wgcT, "w4": w4, "rw4": rw4,
              "bias": bias, "nw": nw, "c": c_h}

    in_maps = []
    for i in range(NCORES):
        rows = slice(i * Bs, (i + 1) * Bs)
        m = dict(shared)
        m["xT"] = _np_mm(np.ascontiguousarray(input[rows].T))
        m["hT"] = _np_mm(np.ascontiguousarray(Hidden_State[rows].T))
        m["rhT"] = _np_mm(np.ascontiguousarray(rHidden_State[rows].T))
        m["cT"] = np.ascontiguousarray(Cell_State[rows].T, dtype=np.float32)
        m["rcT"] = np.ascontiguousarray(rCell_State[rows].T, dtype=np.float32)
        in_maps.append(m)
    return in_maps


def kernel(**inputs):
    nc = _get_nc()
    in_maps = _prep_in_maps(inputs)
    res = bass_utils.run_bass_kernel_spmd(nc, in_maps,
                                          core_ids=list(range(NCORES)))

    def gather(name):
        return np.concatenate([res.results[i][name].T for i in range(NCORES)],
                              axis=0)

    H = gather("HT")
    C_ = gather("CT")
    gc = gather("gcT")
    rH = gather("rHT")
    rC = gather("rCT")
    pred = gather("predT")
    return (H, C_, gc, rH, rC, pred)
